# revision 75
# baseline (speedup 1.0000x reference)
"""DeformableBiomarkerAttention Trainium2 kernel.

Strategy: pure data-parallel over batch (8 batches per NeuronCore, 8 cores).
Per core, a two-group software pipeline (4 batches per 128-row group):
  - consts coalesced into a few DMA blobs (coords-critical head split off
    so the gather index math starts early); trilinear index/weight math in
    ~17 DVE ops, written to be rounding-mode agnostic so CoreSim (trunc)
    and HW (round-to-nearest) agree on the f32->i32 cell indices
  - x host-cast to bf16: halves the gather traffic. Trilinear sampling
    via 8 single-row indirect-DMA gathers per group (multi-row-per-index
    gathers break on real HW); corner weighting = ACT per-partition-scale
    ops + a DVE add-tree (scalar_tensor_tensor has no DVE fast mode, so a
    fused MAC chain would be ~2.4x slower and all on one engine)
  - q projection in fp8 DoubleRow (256-row contraction per PE
    instruction, weights host-scaled x64 to dodge e4m3 subnormals). K/V/O
    stay bf16: their weight-quantization noise reaches the output at full
    sigma (delta-W . s_bar does not average down over attention), which
    would eat the 2e-2 budget; q's noise is damped through the softmax
  - scores for all 4 batches of a group in one 6-matmul chain over a
    [128, 128] psum (only block-diagonal strips are real; head columns
    padded to 32 so the per-batch exp reads are partition-32-aligned, a
    BIR verifier requirement); per-batch ACT exps, softmax without
    max-subtraction (scores bounded far below exp overflow)
  - output in bf16 (halves the biggest DMA stream; host upcasts to f32):
    per batch one [128,4,E] stride-0-source broadcast write, alternating
    between the SP and ACT HWDGE rings; the four 513th rows of a group go
    out as ONE strided [4, E] DMA straight from outfin (each single-row
    DMA otherwise costs ~2.2us of fixed DMA overhead)
  - DMA issue order: consts -> wqt+bioT8+wkt (ACT ring) -> group-0
    gathers -> wvt+wot (SP ring, gated on group-0 gathers) with group-1
    gathers flowing right behind group 0's on the SWDGE ring
"""

import numpy as np
import ml_dtypes

import concourse.bass as bass
import concourse.mybir as mybir
import concourse.tile as tile
from concourse import bass_utils
from concourse.tile_rust import add_dep_helper

F32 = mybir.dt.float32
BF16 = mybir.dt.bfloat16
FP8 = mybir.dt.float8e4
I32 = mybir.dt.int32
ALU = mybir.AluOpType
ACTF = mybir.ActivationFunctionType
DR = mybir.MatmulPerfMode.DoubleRow
WSCALE = 64.0     # fp8 weights are host-scaled by this (subnormal dodge)

E = 768
CH = 6            # number of 128-channel chunks
NB = 32           # points per batch
BPC = 8           # batches per core
FULLN = 513
NCORES = 8
B = 64
ROWS = BPC * NB   # 256 sampled rows per core
NG = 2            # partition groups of 128 rows (4 batches each)
GB = 4            # batches per group
NH = 12           # heads
HD = 64           # head dim

# f32 [128, *] const blob column layout (coords-critical part first so a
# small head DMA unblocks the index math early)
_C_BASE = 0                      # [128, NG, 3]
_C_OFFS = _C_BASE + NG * 3       # [128, NG, 3]
_C_CRB = _C_OFFS + NG * 3        # [128, NG, 8] rowbase + corner offsets
_C_MUL3 = _C_CRB + NG * 8        # [128, 3]
_C_HEAD = _C_MUL3 + 3            # end of the coords-critical head
_C_BQ = _C_HEAD                  # [128, CH]
_C_BK = _C_BQ + CH               # [128, CH]
_C_BV = _C_BK + CH               # [128, CH]
_CF_COLS = _C_BV + CH

# bf16 [128, *] const blob. hsel is padded to 32 head-columns per chunk:
# the merged scores matmul packs (batch, head) onto psum partitions and
# HW APs need 32-aligned partition offsets for the per-batch exp reads.
NHP = 32                         # heads padded to a partition-aligned block
_B_BIOT = 0                      # [128, CH, BPC]
_B_HSEL = _B_BIOT + CH * BPC     # [128, CH, NHP]
_B_IDEN = _B_HSEL + CH * NHP     # [128, 128] identity (bf16: transposes)
_BF_COLS = _B_IDEN + 128

# f32 [GB, *] const blob
_P_CONF = 0                      # [GB, NG]
_P_BO = _P_CONF + NG             # [GB, E]
_PF_COLS = _P_BO + E


def _body(ctx, tc):
    nc = tc.nc

    def inp(name, shape, dt=F32):
        return nc.dram_tensor(name, shape, dt, kind="ExternalInput").ap()

    # ---- DRAM I/O (per-core shard; host prepares these layouts) ----
    x = inp("x", [BPC * FULLN, E], BF16)      # flattened x shard (host-cast)
    cf = inp("cf", [128, _CF_COLS])           # f32 const blob
    cb = inp("cb", [128, _BF_COLS], BF16)     # bf16 const blob
    bsel = inp("bsel", [NH, CH * 128], BF16)  # head-row -> channel broadcast
    onehg = inp("onehg", [GB, GB * 128], BF16)  # group-batch -> 128-row bcast
    pf = inp("pf", [GB, _PF_COLS])            # f32 per-batch blob
    # fp8 is viable only where weight-quantization noise gets damped
    # downstream: q (noise enters via softmax of bounded scores). The V/O
    # (and with sampT shared, K) paths carry noise through at full sigma
    # (delta-W . s_bar is attention-average-invariant), where fp8's 6%
    # would eat the whole 2e-2 error budget.
    wqt = inp("wqt", [128, CH, E], FP8)       # (Wq*64)^T chunked, fp8
    wkt = inp("wkt", [128, CH, E], BF16)      # (Wk @ Ws)^T chunked
    wvt = inp("wvt", [128, CH, E], BF16)      # (Wv @ Ws)^T chunked
    wot = inp("wot", [128, CH, E], BF16)      # Wo^T chunked
    cb8 = inp("cb8", [128, CH * BPC], FP8)    # bio^T chunked, fp8
    out = nc.dram_tensor("out", [BPC * FULLN, E], BF16,
                         kind="ExternalOutput").ap()

    cpool = ctx.enter_context(tc.tile_pool(name="consts", bufs=1))
    wpool = ctx.enter_context(tc.tile_pool(name="weights", bufs=1))
    gpool = ctx.enter_context(tc.tile_pool(name="gather", bufs=1))
    spool = ctx.enter_context(tc.tile_pool(name="small", bufs=1))
    bcpool = ctx.enter_context(tc.tile_pool(name="bcast", bufs=4))
    pp = ctx.enter_context(tc.tile_pool(name="ps", bufs=5, space="PSUM"))
    opp = ctx.enter_context(tc.tile_pool(name="ops", bufs=3, space="PSUM"))

    _psn = [0]

    def psum(shape, pool=None, dt=F32):
        _psn[0] += 1
        return (pool or pp).tile(shape, dt, tag="ps", name=f"ps{_psn[0]}")

    # ---- const blobs (SP queue; the coords-critical head of the f32 blob
    # first: the gathers' index math is the front critical path) ----
    cf_t = cpool.tile([128, _CF_COLS], F32, tag="cf")
    nc.sync.dma_start(out=cf_t[:, :_C_HEAD], in_=cf[:, :_C_HEAD])
    nc.sync.dma_start(out=cf_t[:, _C_HEAD:], in_=cf[:, _C_HEAD:])
    cb_t = cpool.tile([128, _BF_COLS], BF16, tag="cb")
    nc.sync.dma_start(out=cb_t[:], in_=cb[:])
    bsel_t = cpool.tile([NH, CH * 128], BF16, tag="bsel")
    nc.sync.dma_start(out=bsel_t[:], in_=bsel[:])
    oneh_t = cpool.tile([GB, GB * 128], BF16, tag="onehg")
    nc.sync.dma_start(out=oneh_t[:], in_=onehg[:])
    pf_t = cpool.tile([GB, _PF_COLS], F32, tag="pf")
    nc.sync.dma_start(out=pf_t[:], in_=pf[:])

    base_v = cf_t[:, _C_BASE:_C_OFFS].rearrange("p (g c) -> p g c", c=3)
    offs_v = cf_t[:, _C_OFFS:_C_CRB].rearrange("p (g c) -> p g c", c=3)
    crb_v = cf_t[:, _C_CRB:_C_MUL3].rearrange("p (g c) -> p g c", c=8)
    mul3_v = cf_t[:, _C_MUL3:_C_HEAD]                    # [128, 3]
    bq_v = cf_t[:, _C_BQ:_C_BK]
    bk_v = cf_t[:, _C_BK:_C_BV]
    bv_v = cf_t[:, _C_BV:_CF_COLS]
    bioT_v = cb_t[:, _B_BIOT:_B_HSEL].rearrange("p (c b) -> p c b", b=BPC)
    hsel_v = cb_t[:, _B_HSEL:_B_IDEN].rearrange("p (c h) -> p c h", h=NHP)
    iden_v = cb_t[:, _B_IDEN:_BF_COLS]                   # [128, 128] bf16
    conf_v = pf_t[:, _P_CONF:_P_BO]                      # [GB, NG]
    bo_v = pf_t[:, _P_BO:_PF_COLS]                       # [GB, E]

    # ---- coords -> corner row indices + trilinear weights (DVE), both
    # groups at once; op count kept minimal (the DVE sequencer's ~160ns
    # per-instruction dispatch is the front-latency bottleneck).
    # coords order is (x, y, z); flat grid index = 64*z + 8*y + x.
    c_t = spool.tile([128, NG, 3], F32, tag="c")
    nc.vector.tensor_add(out=c_t[:], in0=base_v, in1=offs_v)
    nc.vector.tensor_scalar(out=c_t[:], in0=c_t[:], scalar1=1.0,
                            scalar2=-1.0, op0=ALU.min, op1=ALU.max)
    # i_shift = (c + 6/7) * 3.5 = i - 0.5 where i = (c + 1) * 3.5;
    # floor(i) == round_or_trunc(i - 0.5) for i in [0, 7] (both rounding
    # modes give a valid (i0, w) pair; w absorbs the edge cases)
    ish_t = spool.tile([128, NG, 3], F32, tag="ish")
    nc.vector.tensor_scalar(out=ish_t[:], in0=c_t[:],
                            scalar1=6.0 / 7.0, scalar2=3.5,
                            op0=ALU.add, op1=ALU.mult)
    i0i_t = spool.tile([128, NG, 3], I32, tag="i0i")
    nc.vector.tensor_copy(out=i0i_t[:], in_=ish_t[:])
    i0f_t = spool.tile([128, NG, 3], F32, tag="i0f")
    nc.vector.tensor_copy(out=i0f_t[:], in_=i0i_t[:])
    # f32->i32 convert rounding differs between HW (round-to-nearest) and
    # CoreSim (truncate). Make i0 = floor(i) under either mode:
    # d = i - cvt(i - 0.5); i0 += (d >= 1).
    d_t = spool.tile([128, NG, 3], F32, tag="dcorr")
    nc.vector.scalar_tensor_tensor(out=d_t[:], in0=ish_t[:], scalar=0.5,
                                   in1=i0f_t[:], op0=ALU.add,
                                   op1=ALU.subtract)
    nc.vector.tensor_scalar(out=d_t[:], in0=d_t[:], scalar1=1.0,
                            scalar2=None, op0=ALU.is_ge)
    nc.vector.tensor_add(out=i0f_t[:], in0=i0f_t[:], in1=d_t[:])
    nc.vector.tensor_scalar(out=i0f_t[:], in0=i0f_t[:], scalar1=6.0,
                            scalar2=0.0, op0=ALU.min, op1=ALU.max)
    # interleaved (1-w, w) pairs: wall[..., 1] = w = (i_shift + 0.5) - i0,
    # wall[..., 0] = 1 - w
    wall_t = spool.tile([128, NG, 3, 2], F32, tag="wall")
    nc.vector.scalar_tensor_tensor(out=wall_t[:, :, :, 1], in0=ish_t[:],
                                   scalar=0.5, in1=i0f_t[:],
                                   op0=ALU.add, op1=ALU.subtract)
    nc.vector.tensor_scalar(out=wall_t[:, :, :, 0], in0=wall_t[:, :, :, 1],
                            scalar1=-1.0, scalar2=1.0,
                            op0=ALU.mult, op1=ALU.add)
    # base row of the point's cell, then all 8 corner rows in one add
    # (crb holds rowbase + 64*cz + 8*cy + xb per corner, built on host)
    pr_t = spool.tile([128, NG, 3], F32, tag="pr")
    nc.vector.tensor_mul(out=pr_t[:], in0=i0f_t[:],
                         in1=mul3_v.unsqueeze(1).to_broadcast([128, NG, 3]))
    ib_t = spool.tile([128, NG, 1], F32, tag="ib")
    nc.vector.reduce_sum(out=ib_t[:], in_=pr_t[:], axis=mybir.AxisListType.X)
    idx8f_t = spool.tile([128, NG, 8], F32, tag="idx8f")
    nc.vector.tensor_add(out=idx8f_t[:],
                         in0=ib_t[:].to_broadcast([128, NG, 8]), in1=crb_v)
    idx_t = spool.tile([128, NG, 8], I32, tag="idx")
    nc.vector.tensor_copy(out=idx_t[:], in_=idx8f_t[:])
    # corner weights wc[..., (cz, cy), xb] = zsel * ysel * xsel via two
    # outer products over the interleaved pairs
    yz_t = spool.tile([128, NG, 2, 2], F32, tag="yz")
    nc.vector.tensor_mul(
        out=yz_t[:],
        in0=wall_t[:, :, 2, :].unsqueeze(3).to_broadcast([128, NG, 2, 2]),
        in1=wall_t[:, :, 1, :].unsqueeze(2).to_broadcast([128, NG, 2, 2]))
    wc_t = spool.tile([128, NG, 4, 2], F32, tag="wc")
    nc.vector.tensor_mul(
        out=wc_t[:],
        in0=yz_t[:].rearrange("p g a b -> p g (a b)").unsqueeze(3)
            .to_broadcast([128, NG, 4, 2]),
        in1=wall_t[:, :, 0, :].unsqueeze(2).to_broadcast([128, NG, 4, 2]))
    wc_t = wc_t[:].rearrange("p g a b -> p g (a b)")

    # ---- DMA ordering plan (everything contends for the same DMA
    # engines, so the issue order is sequenced with explicit deps):
    #   consts -> wkt+wqt -> group-0 gathers -> wvt+wot -> group-1
    #   gathers -> output writes.
    # wkt/wqt (needed first: K pass, scores) load during the dead time
    # before the gathers' indices are computed; wvt/wot wait until the
    # group-0 gathers are through; group 1's gathers yield to wvt/wot. ----
    w_tiles = {}
    w_dmas = {}
    bioT8 = None
    for name, ap, dt_ in (("wqt", wqt, FP8), ("wkt", wkt, BF16),
                          ("wvt", wvt, BF16), ("wot", wot, BF16)):
        t = wpool.tile([128, CH, E], dt_, tag=name)
        eng = nc.scalar if name in ("wkt", "wqt") else nc.sync
        d0 = eng.dma_start(out=t[:, 0:3, :], in_=ap[:, 0:3, :])
        d1 = eng.dma_start(out=t[:, 3:6, :], in_=ap[:, 3:6, :])
        w_tiles[name] = t
        w_dmas[name] = (d0, d1)
        if name == "wqt":
            # q's other operand, right behind wqt on the ACT ring
            bioT8 = cpool.tile([128, CH * BPC], FP8, tag="cb8")
            nc.scalar.dma_start(out=bioT8[:], in_=cb8[:])
    bioT8_v = bioT8[:].rearrange("p (c b) -> p c b", b=BPC)

    # ---- all 16 single-row gathers issued up front (qPoolDynamic) ----
    corner_tiles = {}
    gather_insts = {}
    for g in range(NG):
        for c8 in range(8):
            pt = gpool.tile([128, E], BF16, tag=f"corner{g}{c8}")
            gi = nc.gpsimd.indirect_dma_start(
                out=pt[:], out_offset=None, in_=x[:],
                in_offset=bass.IndirectOffsetOnAxis(
                    ap=idx_t[:, g, c8:c8 + 1], axis=0),
            )
            corner_tiles[(g, c8)] = pt
            gather_insts[(g, c8)] = gi

    # wvt/wot yield to group 0's gathers (the SP HWDGE ring is otherwise
    # free then, and the gathers are the front critical path). Group 1's
    # gathers flow right behind group 0's on the SWDGE rings — with x in
    # bf16 there is enough HBM headroom to overlap them with wvt/wot.
    for name in ("wvt", "wot"):
        for d in w_dmas[name]:
            add_dep_helper(d.ins, gather_insts[(0, 5)].ins,
                           reason="late weights wait for group-0 gathers")

    # ---- trilinear corner accumulate: per-corner in-place scales + DVE
    # add-tree. Group 0's scales run on ACT (its window is before the
    # drain/exp traffic; activation takes a per-partition AP scale);
    # group 1's run on DVE in 2x mode (443ns tensor_scalar) because by
    # then ACT is congested with drains. bf16 throughout (~1e-3 extra
    # rel err from 4 roundings).
    def mac_chain(acc, g, scale_eng):
        cs = []
        for c8 in range(8):
            ct = corner_tiles[(g, c8)]
            if scale_eng == "act":
                nc.scalar.activation(out=ct[:], in_=ct[:],
                                     func=ACTF.Identity, bias=0.0,
                                     scale=wc_t[:, g, c8:c8 + 1])
            else:
                nc.vector.tensor_scalar(out=ct[:], in0=ct[:],
                                        scalar1=wc_t[:, g, c8:c8 + 1],
                                        scalar2=None, op0=ALU.mult)
            cs.append(ct)
        for a, b in ((0, 1), (2, 3), (4, 5), (6, 7), (0, 2), (4, 6)):
            nc.vector.tensor_add(out=cs[a][:], in0=cs[a][:], in1=cs[b][:])
        nc.vector.tensor_add(out=acc[:], in0=cs[0][:], in1=cs[4][:])

    acc0 = spool.tile([128, E], BF16, tag="acc0", name="acc0")
    mac_chain(acc0, 0, "act")
    acc1 = spool.tile([128, E], BF16, tag="acc1", name="acc1")
    acc_g = [acc0, acc1]

    # ---- q projection (all 8 batches): qT[co] = (Wq @ bio^T + bq) / 8.
    # Emitted first in the PE stream: PE is idle until the transposes are
    # ready, and q only depends on wqt + the bioT const. ----
    qT = []
    for co in range(CH):
        ps = psum([128, BPC])
        for t8 in range(CH // 2):
            nc.tensor.matmul(
                out=ps[:],
                lhsT=w_tiles["wqt"][:, 2 * t8:2 * t8 + 2,
                                    128 * co:128 * (co + 1)],
                rhs=bioT8_v[:, 2 * t8:2 * t8 + 2, :],
                start=(t8 == 0), stop=(t8 == CH // 2 - 1), perf_mode=DR)
        qt = cpool.tile([128, BPC], BF16, tag=f"qT{co}", name=f"qT{co}")
        nc.scalar.activation(out=qt[:], in_=ps[:], func=ACTF.Identity,
                             bias=bq_v[:, co:co + 1], scale=0.125 / WSCALE)
        qT.append(qt)

    # qexp after group 0's MAC in the DVE stream (scores need it later);
    # head columns padded to 32 so the scores psum blocks land 32-aligned
    qexp = []
    for ci in range(CH):
        qe = cpool.tile([128, BPC, NHP], BF16, tag=f"qexp{ci}",
                        name=f"qexp{ci}")
        nc.vector.tensor_mul(
            out=qe[:],
            in0=qT[ci][:].unsqueeze(2).to_broadcast([128, BPC, NHP]),
            in1=hsel_v[:, ci, :].unsqueeze(1).to_broadcast([128, BPC, NHP]))
        qexp.append(qe)

    boc_g = []
    for gg in range(NG):
        bc_ = spool.tile([GB, E], F32, tag=f"boc{gg}", name=f"boc{gg}")
        nc.vector.tensor_scalar(out=bc_[:], in0=bo_v,
                                scalar1=conf_v[:, gg:gg + 1],
                                scalar2=None, op0=ALU.mult)
        boc_g.append(bc_)

    # ---- per-group pipeline ----
    for g in range(NG):
        acc = acc_g[g]

        # transpose to channel-major bf16 (PSUM drain copies split between
        # ACT and DVE so neither serializes the chain)
        sampT = spool.tile([128, CH, 128], BF16, tag=f"sampT{g}",
                           name=f"sampT{g}")
        for ci in range(CH):
            ps = psum([128, 128], dt=BF16)
            nc.tensor.transpose(
                out=ps[:], in_=acc[:, 128 * ci:128 * (ci + 1)],
                identity=iden_v)
            if ci % 2 == 0:
                nc.scalar.copy(out=sampT[:, ci, :], in_=ps[:])
            else:
                nc.vector.tensor_copy(out=sampT[:, ci, :], in_=ps[:])

        # K / V projections (weights pre-folded with sample_proj)
        def proj_pass(wname, bias_v, out_tag):
            outs = []
            for co in range(CH):
                ps = psum([128, 128])
                for ci in range(CH):
                    nc.tensor.matmul(
                        out=ps[:],
                        lhsT=w_tiles[wname][:, ci, 128 * co:128 * (co + 1)],
                        rhs=sampT[:, ci, :],
                        start=(ci == 0), stop=(ci == CH - 1))
                o = spool.tile([128, 128], BF16, tag=f"{out_tag}{g}{co}",
                               name=f"{out_tag}{g}{co}")
                if co % 2 == 0:
                    nc.scalar.activation(out=o[:], in_=ps[:],
                                         func=ACTF.Identity,
                                         bias=bias_v[:, co:co + 1], scale=1.0)
                else:
                    nc.vector.tensor_scalar(out=o[:], in0=ps[:],
                                            scalar1=bias_v[:, co:co + 1],
                                            scalar2=None, op0=ALU.add)
                outs.append(o)
            return outs

        kT = proj_pass("wkt", bk_v, "kT")

        # scores, all 4 batches in one matmul chain: lhsT packs the group's
        # (batch, head) q columns -> psum [48, 128]; only the block-diagonal
        # (b == b') strips are real, the off-diagonal 3/4 is discarded. 6
        # matmuls replace 24 tiny ones (PE instruction overhead dominated).
        sc_ps = psum([GB * NHP, GB * NB])
        qsl = slice(GB * g, GB * (g + 1))
        for ci in range(CH):
            nc.tensor.matmul(
                out=sc_ps[:],
                lhsT=qexp[ci][:, qsl, :].rearrange("p b h -> p (b h)"),
                rhs=kT[ci][:], start=(ci == 0), stop=(ci == CH - 1))

        # softmax over points. Scores here are bounded (|s| < ~1: q is
        # pre-scaled by 1/8 and both operands are O(0.3)-scale random
        # projections), so exp() is computed without the max-subtraction.
        ex_t = spool.tile([NH, GB, NB], F32, tag=f"ex{g}", name=f"ex{g}")
        for b in range(GB):
            nc.scalar.activation(out=ex_t[:, b, :],
                                 in_=sc_ps[NHP * b:NHP * b + NH,
                                           NB * b:NB * (b + 1)],
                                 func=ACTF.Exp)
        s_t = spool.tile([NH, GB, 1], F32, tag=f"sm{g}", name=f"sm{g}")
        nc.vector.reduce_sum(out=s_t[:], in_=ex_t[:],
                             axis=mybir.AxisListType.X)
        r_t = spool.tile([NH, GB], F32, tag=f"rc{g}", name=f"rc{g}")
        nc.vector.reciprocal(out=r_t[:], in_=s_t[:, :, 0])
        at_t = spool.tile([NH, GB, NB], BF16, tag=f"attn{g}", name=f"attn{g}")
        nc.vector.tensor_mul(out=at_t[:], in0=ex_t[:],
                             in1=r_t[:].unsqueeze(2).to_broadcast(
                                 [NH, GB, NB]))

        # V projection after the scores/softmax (its weights arrive later)
        vT = proj_pass("wvt", bv_v, "vT")

        if g == 0:
            # group 1's corner accumulation, emitted after group 0's V
            # drains: its gathers are done by then, and emitting later
            # strands acc1 behind group 0's full ACT drain traffic
            mac_chain(acc1, 1, "act")

        # broadcast attn rows to channel layout; ctx reduction (DVE reads
        # the PSUM product input directly; no staging copy)
        ctxF = spool.tile([128, CH, GB], F32, tag=f"ctxF{g}", name=f"ctxF{g}")
        for ci in range(CH):
            ps = psum([128, GB * NB])
            nc.tensor.matmul(
                out=ps[:], lhsT=bsel_t[:, 128 * ci:128 * (ci + 1)],
                rhs=at_t[:], start=True, stop=True)
            prod = spool.tile([128, GB, NB], F32, tag=f"prod{g}{ci}",
                              name=f"prod{g}{ci}")
            nc.vector.tensor_mul(
                out=prod[:],
                in0=vT[ci][:].rearrange("p (b n) -> p b n", n=NB),
                in1=ps[:].rearrange("p (b n) -> p b n", n=NB))
            nc.vector.reduce_sum(out=ctxF[:, ci, :].unsqueeze(2),
                                 in_=prod[:], axis=mybir.AxisListType.X)
        ctxT = spool.tile([128, CH, GB], BF16, tag=f"ctxT{g}", name=f"ctxT{g}")
        nc.vector.tensor_copy(out=ctxT[:], in_=ctxF[:])

        # out projection + bias + confidence: outfin = ps*conf + bo*conf
        outfin = spool.tile([GB, E], BF16, tag=f"outfin{g}", name=f"outfin{g}")
        for half in range(2):
            sl = slice(384 * half, 384 * (half + 1))
            ps = psum([GB, 384], opp)
            for ci in range(CH):
                nc.tensor.matmul(
                    out=ps[:], lhsT=ctxT[:, ci, :],
                    rhs=w_tiles["wot"][:, ci, sl],
                    start=(ci == 0), stop=(ci == CH - 1))
            nc.vector.scalar_tensor_tensor(
                out=outfin[:, sl], in0=ps[:],
                scalar=conf_v[:, g:g + 1],
                in1=boc_g[g][:][:, sl],
                op0=ALU.mult, op1=ALU.add)

        # broadcast each batch row to 128 partitions; write 512 rows per
        # batch, alternating between the SP and ACT HWDGE rings so the four
        # transfers pipeline two-wide. The 513th rows of the whole group go
        # out as ONE [4, E] strided DMA straight from outfin (saves four
        # fixed-cost-dominated single-row DMAs).
        for b in range(GB):
            bb = GB * g + b
            bt = bcpool.tile([128, E], BF16, tag="bt", name=f"bt{bb}")
            for half in range(2):
                sl = slice(384 * half, 384 * (half + 1))
                ps = psum([128, 384], opp)
                nc.tensor.matmul(
                    out=ps[:], lhsT=oneh_t[:, 128 * b:128 * (b + 1)],
                    rhs=outfin[:, sl], start=True, stop=True)
                if (b + half) % 2 == 0:
                    nc.scalar.copy(out=bt[:, sl], in_=ps[:])
                else:
                    nc.vector.tensor_copy(out=bt[:, sl], in_=ps[:])
            r0 = FULLN * bb
            dst = out[r0:r0 + 512, :].rearrange("(p f) e -> p f e", f=4)
            src = bt[:].unsqueeze(1).to_broadcast([128, 4, E])
            eng = nc.sync if b % 2 == 0 else nc.scalar
            if b == 0:
                # the group's first write: two half-width DMAs, each gated
                # only on its own staging copy, so the first transfer's
                # HWDGE issue overlaps the second half's copy
                for half in range(2):
                    sl = slice(384 * half, 384 * (half + 1))
                    eng.dma_start(out=dst[:, :, sl], in_=src[:, :, sl])
            else:
                eng.dma_start(out=dst, in_=src)
        tail_dst = out.rearrange("(b r) e -> b r e", r=FULLN)[
            GB * g:GB * (g + 1), FULLN - 1, :]
        nc.sync.dma_start(out=tail_dst, in_=outfin[:])


_NO_SPLIT_TYPES = {"InstUnconditionalBranch", "InstConditionalBranch"}


def _split_waits(nc, max_waits=1):
    # walrus (CoreV3) accepts only one sync-wait command per compute
    # instruction; move extra waits onto injected same-engine NoOps placed
    # immediately before the instruction (semantics unchanged).
    import bass_rust
    k = 0
    for fn in nc.m.functions:
        for bb in fn.blocks:
            insts = bb.instructions
            i = 0
            while i < len(insts):
                inst = insts[i]
                si = inst.sync_info
                if (type(inst).__name__ not in _NO_SPLIT_TYPES
                        and si is not None
                        and si.on_wait and len(si.on_wait) > max_waits):
                    waits = list(si.on_wait)
                    extra, keep = waits[:-max_waits], waits[-max_waits:]
                    for w in extra:
                        k += 1
                        nop = bass_rust.InstNoOp(name=f"I-wsplit-{k}",
                                                 engine=inst.engine,
                                                 ins=[], outs=[])
                        nop.sync_info = bass_rust.SyncInfo(on_wait=[w],
                                                           on_update=[])
                        insts.insert(i, nop)
                        i += 1
                    inst.sync_info = bass_rust.SyncInfo(
                        on_wait=keep, on_update=list(si.on_update or []))
                i += 1
    return k


def build(split=True):
    from contextlib import ExitStack

    # 48KB SWDGE descriptor carveout: all 16 indirect gathers (129 descs
    # each) fit in flight at once; the default 16KB ring caps ~8 and
    # trickles the second group's gathers behind the first's completions.
    nc = bass.Bass("TRN2", debug=False, num_devices=NCORES,
                   dynamic_dma_scratch_size=16384)
    with tile.TileContext(nc) as tc, ExitStack() as es:
        _body(es, tc)
    if split:
        # needed for the walrus compile; CoreSim can't replay injected nops
        _split_waits(nc)
    return nc


def host_prep(inputs):
    """Build per-core in_maps from full inputs (layout marshalling + weight
    folding/casting only)."""
    x = np.ascontiguousarray(inputs["x"], dtype=np.float32)
    bio = np.ascontiguousarray(inputs["bio_embed"], dtype=np.float32)
    base = np.ascontiguousarray(inputs["base_coords"], dtype=np.float32)
    offsets = np.ascontiguousarray(inputs["offsets"], dtype=np.float32)
    confidence = np.ascontiguousarray(inputs["confidence"], dtype=np.float32)
    wsp = np.asarray(inputs["sample_proj_w"], dtype=np.float32)
    bsp = np.asarray(inputs["sample_proj_b"], dtype=np.float32)
    win = np.asarray(inputs["in_proj_w"], dtype=np.float32)
    bin_ = np.asarray(inputs["in_proj_b"], dtype=np.float32)
    wout = np.asarray(inputs["out_proj_w"], dtype=np.float32)
    bout = np.asarray(inputs["out_proj_b"], dtype=np.float32)

    # fold sample_proj into Wk / Wv (exact algebra, done in f64 on host)
    wk, wv = win[E:2 * E], win[2 * E:]
    bkf = wk @ bsp + bin_[E:2 * E]
    bvf = wv @ bsp + bin_[2 * E:]
    wks = (wk.astype(np.float64) @ wsp.astype(np.float64)).astype(np.float32)
    wvs = (wv.astype(np.float64) @ wsp.astype(np.float64)).astype(np.float32)

    def chunkT(w, dt=ml_dtypes.bfloat16):  # [E, E] -> [128, CH, E] of w^T
        return np.ascontiguousarray(
            w.T.reshape(CH, 128, E).transpose(1, 0, 2)).astype(dt)

    _FP8 = ml_dtypes.float8_e4m3

    def chunkT8(w):  # fp8, host-scaled by WSCALE to dodge e4m3 subnormals
        return chunkT(w * WSCALE, _FP8)

    def chunkb(v):  # [E] -> [128, CH]
        return np.ascontiguousarray(v.reshape(CH, 128).T)

    # f32 [128, *] const blob
    cfb = np.zeros((128, _CF_COLS), np.float32)
    cfb[:, _C_BASE:_C_OFFS] = np.tile(base, (BPC, 1)).reshape(
        NG, 128, 3).transpose(1, 0, 2).reshape(128, NG * 3)
    rowb = ((np.arange(ROWS) // NB) * FULLN + 1.0).astype(
        np.float32).reshape(NG, 128).T  # [128, NG]
    coff = np.array([64 * cz + 8 * cy + xb
                     for (cz, cy) in ((0, 0), (0, 1), (1, 0), (1, 1))
                     for xb in (0, 1)], np.float32)  # [8]
    cfb[:, _C_CRB:_C_MUL3] = (rowb[:, :, None] + coff[None, None, :]).reshape(
        128, NG * 8)
    cfb[:, _C_MUL3:_C_HEAD] = np.tile(
        np.array([1.0, 8.0, 64.0], np.float32), (128, 1))
    cfb[:, _C_BQ:_C_BK] = chunkb(bin_[:E] * 0.125)
    cfb[:, _C_BK:_C_BV] = chunkb(bkf)
    cfb[:, _C_BV:_CF_COLS] = chunkb(bvf)

    # bf16 [128, *] const blob (bioT filled per core below)
    cbb = np.zeros((128, _BF_COLS), np.float32)
    hsel = np.zeros((128, CH, NHP), np.float32)
    for ch in range(CH):
        for p in range(128):
            hsel[p, ch, (ch * 128 + p) // HD] = 1.0
    cbb[:, _B_HSEL:_B_IDEN] = hsel.reshape(128, CH * NHP)
    cbb[:, _B_IDEN:_BF_COLS] = np.eye(128, dtype=np.float32)

    bsel = np.zeros((NH, CH * 128), np.float32)
    for ch in range(CH):
        for j in range(128):
            bsel[(ch * 128 + j) // HD, ch * 128 + j] = 1.0
    oneh = np.zeros((GB, GB * 128), np.float32)
    for b in range(GB):
        oneh[b, 128 * b:128 * (b + 1)] = 1.0

    consts = {
        "wqt": chunkT8(win[:E]),
        "wkt": chunkT(wks),
        "wvt": chunkT(wvs),
        "wot": chunkT(wout),
        "cf": cfb,
        "bsel": bsel.astype(ml_dtypes.bfloat16),
        "onehg": oneh.astype(ml_dtypes.bfloat16),
    }

    in_maps = []
    for c in range(NCORES):
        bsl = slice(BPC * c, BPC * (c + 1))
        bio_c = bio[bsl]  # [8, 768]
        m = dict(consts)
        m["x"] = x[bsl].reshape(BPC * FULLN, E).astype(ml_dtypes.bfloat16)
        cfc = cfb.copy()
        cfc[:, _C_OFFS:_C_CRB] = offsets[bsl].reshape(
            NG, 128, 3).transpose(1, 0, 2).reshape(128, NG * 3)
        m["cf"] = cfc
        cbc = cbb.copy()
        bioT = bio_c.T.reshape(CH, 128, BPC).transpose(
            1, 0, 2).reshape(128, CH * BPC)
        cbc[:, _B_BIOT:_B_HSEL] = bioT
        m["cb"] = cbc.astype(ml_dtypes.bfloat16)
        m["cb8"] = bioT.astype(_FP8)
        pfb = np.zeros((GB, _PF_COLS), np.float32)
        pfb[:, _P_CONF:_P_BO] = confidence[bsl].reshape(NG, GB).T
        pfb[:, _P_BO:_PF_COLS] = bout[None, :]
        m["pf"] = pfb
        in_maps.append(m)
    return in_maps


_NC = None


def kernel(**inputs):
    global _NC
    if _NC is None:
        _NC = build()
    in_maps = host_prep(inputs)
    res = bass_utils.run_bass_kernel_spmd(_NC, in_maps,
                                          core_ids=list(range(NCORES)))
    outs = [res.results[c]["out"].reshape(BPC, FULLN, E).astype(np.float32)
            for c in range(NCORES)]
    return np.concatenate(outs, axis=0)



# revision 77
# speedup vs baseline: 1.1320x; 1.1320x over previous
"""DeformableBiomarkerAttention Trainium2 kernel.

Strategy: pure data-parallel over batch (8 batches per NeuronCore, 8 cores).
Per core, a two-group software pipeline (4 batches per 128-row group):
  - consts coalesced into a few DMA blobs (coords-critical head split off
    so the gather index math starts early); trilinear index/weight math in
    ~17 DVE ops, written to be rounding-mode agnostic so CoreSim (trunc)
    and HW (round-to-nearest) agree on the f32->i32 cell indices
  - x host-cast to bf16: halves the gather traffic. Trilinear sampling
    via 8 single-row indirect-DMA gathers per group (multi-row-per-index
    gathers break on real HW); corner weighting = ACT per-partition-scale
    ops + a DVE add-tree (scalar_tensor_tensor has no DVE fast mode, so a
    fused MAC chain would be ~2.4x slower and all on one engine)
  - q projection in fp8 DoubleRow (256-row contraction per PE
    instruction, weights host-scaled x64 to dodge e4m3 subnormals). K/V/O
    stay bf16: their weight-quantization noise reaches the output at full
    sigma (delta-W . s_bar does not average down over attention), which
    would eat the 2e-2 budget; q's noise is damped through the softmax
  - scores for all 4 batches of a group in one 6-matmul chain over a
    [128, 128] psum (only block-diagonal strips are real; head columns
    padded to 32 so the per-batch exp reads are partition-32-aligned, a
    BIR verifier requirement); per-batch ACT exps, softmax without
    max-subtraction (scores bounded far below exp overflow)
  - output in bf16 (halves the biggest DMA stream; host upcasts to f32):
    per batch one [128,4,E] stride-0-source broadcast write, alternating
    between the SP and ACT HWDGE rings; the four 513th rows of a group go
    out as ONE strided [4, E] DMA straight from outfin (each single-row
    DMA otherwise costs ~2.2us of fixed DMA overhead)
  - DMA issue order: consts -> wqt+bioT8+wkt (ACT ring) -> group-0
    gathers -> wvt+wot (SP ring, gated on group-0 gathers) with group-1
    gathers flowing right behind group 0's on the SWDGE ring
"""

import numpy as np
import ml_dtypes

import concourse.bass as bass
import concourse.mybir as mybir
import concourse.tile as tile
from concourse import bass_utils
from concourse.tile_rust import add_dep_helper

F32 = mybir.dt.float32
BF16 = mybir.dt.bfloat16
FP8 = mybir.dt.float8e4
I32 = mybir.dt.int32
ALU = mybir.AluOpType
ACTF = mybir.ActivationFunctionType
DR = mybir.MatmulPerfMode.DoubleRow
WSCALE = 64.0     # fp8 weights are host-scaled by this (subnormal dodge)

E = 768
CH = 6            # number of 128-channel chunks
NB = 32           # points per batch
BPC = 8           # batches per core
FULLN = 513
NCORES = 8
B = 64
ROWS = BPC * NB   # 256 sampled rows per core
NG = 2            # partition groups of 128 rows (4 batches each)
GB = 4            # batches per group
NH = 12           # heads
HD = 64           # head dim

# f32 [128, *] const blob column layout (coords-critical part first so a
# small head DMA unblocks the index math early)
_C_BASE = 0                      # [128, NG, 3]
_C_OFFS = _C_BASE + NG * 3       # [128, NG, 3]
_C_CRB = _C_OFFS + NG * 3        # [128, NG, 8] rowbase + corner offsets
_C_MUL3 = _C_CRB + NG * 8        # [128, 3]
_C_HEAD = _C_MUL3 + 3            # end of the coords-critical head
_C_BQ = _C_HEAD                  # [128, CH]
_C_BK = _C_BQ + CH               # [128, CH]
_C_BV = _C_BK + CH               # [128, CH]
_CF_COLS = _C_BV + CH

# bf16 [128, *] const blob. hsel is padded to 32 head-columns per chunk:
# the merged scores matmul packs (batch, head) onto psum partitions and
# HW APs need 32-aligned partition offsets for the per-batch exp reads.
NHP = 32                         # heads padded to a partition-aligned block
_B_BIOT = 0                      # [128, CH, BPC]
_B_HSEL = _B_BIOT + CH * BPC     # [128, CH, NHP]
_B_IDEN = _B_HSEL + CH * NHP     # [128, 128] identity (bf16: transposes)
_BF_COLS = _B_IDEN + 128

# f32 [GB, *] const blob
_P_CONF = 0                      # [GB, NG]
_P_BO = _P_CONF + NG             # [GB, E]
_PF_COLS = _P_BO + E


def _body(ctx, tc):
    nc = tc.nc

    def inp(name, shape, dt=F32):
        return nc.dram_tensor(name, shape, dt, kind="ExternalInput").ap()

    # ---- DRAM I/O (per-core shard; host prepares these layouts) ----
    x = inp("x", [BPC * FULLN, E], BF16)      # flattened x shard (host-cast)
    cf = inp("cf", [128, _CF_COLS])           # f32 const blob
    cb = inp("cb", [128, _BF_COLS], BF16)     # bf16 const blob
    bsel = inp("bsel", [NH, CH * 128], BF16)  # head-row -> channel broadcast
    onehg = inp("onehg", [GB, GB * 128], BF16)  # group-batch -> 128-row bcast
    pf = inp("pf", [GB, _PF_COLS])            # f32 per-batch blob
    # fp8 is viable only where weight-quantization noise gets damped
    # downstream: q (noise enters via softmax of bounded scores). The V/O
    # (and with sampT shared, K) paths carry noise through at full sigma
    # (delta-W . s_bar is attention-average-invariant), where fp8's 6%
    # would eat the whole 2e-2 error budget.
    wqt = inp("wqt", [128, CH, E], FP8)       # (Wq*64)^T chunked, fp8
    wkt = inp("wkt", [128, CH, E], BF16)      # (Wk @ Ws)^T chunked
    wvt = inp("wvt", [128, CH, E], BF16)      # (Wv @ Ws)^T chunked
    wot = inp("wot", [128, CH, E], BF16)      # Wo^T chunked
    cb8 = inp("cb8", [128, CH * BPC], FP8)    # bio^T chunked, fp8
    out = nc.dram_tensor("out", [BPC * FULLN, E], BF16,
                         kind="ExternalOutput").ap()

    cpool = ctx.enter_context(tc.tile_pool(name="consts", bufs=1))
    wpool = ctx.enter_context(tc.tile_pool(name="weights", bufs=1))
    gpool = ctx.enter_context(tc.tile_pool(name="gather", bufs=1))
    spool = ctx.enter_context(tc.tile_pool(name="small", bufs=1))
    bcpool = ctx.enter_context(tc.tile_pool(name="bcast", bufs=4))
    pp = ctx.enter_context(tc.tile_pool(name="ps", bufs=5, space="PSUM"))
    opp = ctx.enter_context(tc.tile_pool(name="ops", bufs=3, space="PSUM"))

    _psn = [0]

    def psum(shape, pool=None, dt=F32):
        _psn[0] += 1
        return (pool or pp).tile(shape, dt, tag="ps", name=f"ps{_psn[0]}")

    # ---- const blobs (SP queue; the coords-critical head of the f32 blob
    # first: the gathers' index math is the front critical path) ----
    cf_t = cpool.tile([128, _CF_COLS], F32, tag="cf")
    nc.sync.dma_start(out=cf_t[:, :_C_HEAD], in_=cf[:, :_C_HEAD])
    nc.sync.dma_start(out=cf_t[:, _C_HEAD:], in_=cf[:, _C_HEAD:])
    cb_t = cpool.tile([128, _BF_COLS], BF16, tag="cb")
    nc.sync.dma_start(out=cb_t[:], in_=cb[:])
    bsel_t = cpool.tile([NH, CH * 128], BF16, tag="bsel")
    nc.sync.dma_start(out=bsel_t[:], in_=bsel[:])
    oneh_t = cpool.tile([GB, GB * 128], BF16, tag="onehg")
    nc.sync.dma_start(out=oneh_t[:], in_=onehg[:])
    pf_t = cpool.tile([GB, _PF_COLS], F32, tag="pf")
    nc.sync.dma_start(out=pf_t[:], in_=pf[:])

    base_v = cf_t[:, _C_BASE:_C_OFFS].rearrange("p (g c) -> p g c", c=3)
    offs_v = cf_t[:, _C_OFFS:_C_CRB].rearrange("p (g c) -> p g c", c=3)
    crb_v = cf_t[:, _C_CRB:_C_MUL3].rearrange("p (g c) -> p g c", c=8)
    mul3_v = cf_t[:, _C_MUL3:_C_HEAD]                    # [128, 3]
    bq_v = cf_t[:, _C_BQ:_C_BK]
    bk_v = cf_t[:, _C_BK:_C_BV]
    bv_v = cf_t[:, _C_BV:_CF_COLS]
    bioT_v = cb_t[:, _B_BIOT:_B_HSEL].rearrange("p (c b) -> p c b", b=BPC)
    hsel_v = cb_t[:, _B_HSEL:_B_IDEN].rearrange("p (c h) -> p c h", h=NHP)
    iden_v = cb_t[:, _B_IDEN:_BF_COLS]                   # [128, 128] bf16
    conf_v = pf_t[:, _P_CONF:_P_BO]                      # [GB, NG]
    bo_v = pf_t[:, _P_BO:_PF_COLS]                       # [GB, E]

    # ---- coords -> corner row indices + trilinear weights (DVE), both
    # groups at once; op count kept minimal (the DVE sequencer's ~160ns
    # per-instruction dispatch is the front-latency bottleneck).
    # coords order is (x, y, z); flat grid index = 64*z + 8*y + x.
    c_t = spool.tile([128, NG, 3], F32, tag="c")
    nc.vector.tensor_add(out=c_t[:], in0=base_v, in1=offs_v)
    nc.vector.tensor_scalar(out=c_t[:], in0=c_t[:], scalar1=1.0,
                            scalar2=-1.0, op0=ALU.min, op1=ALU.max)
    # i_shift = (c + 6/7) * 3.5 = i - 0.5 where i = (c + 1) * 3.5;
    # floor(i) == round_or_trunc(i - 0.5) for i in [0, 7] (both rounding
    # modes give a valid (i0, w) pair; w absorbs the edge cases)
    ish_t = spool.tile([128, NG, 3], F32, tag="ish")
    nc.vector.tensor_scalar(out=ish_t[:], in0=c_t[:],
                            scalar1=6.0 / 7.0, scalar2=3.5,
                            op0=ALU.add, op1=ALU.mult)
    i0i_t = spool.tile([128, NG, 3], I32, tag="i0i")
    nc.vector.tensor_copy(out=i0i_t[:], in_=ish_t[:])
    i0f_t = spool.tile([128, NG, 3], F32, tag="i0f")
    nc.vector.tensor_copy(out=i0f_t[:], in_=i0i_t[:])
    # f32->i32 convert rounding differs between HW (round-to-nearest) and
    # CoreSim (truncate). Make i0 = floor(i) under either mode:
    # d = i - cvt(i - 0.5); i0 += (d >= 1).
    d_t = spool.tile([128, NG, 3], F32, tag="dcorr")
    nc.vector.scalar_tensor_tensor(out=d_t[:], in0=ish_t[:], scalar=0.5,
                                   in1=i0f_t[:], op0=ALU.add,
                                   op1=ALU.subtract)
    nc.vector.tensor_scalar(out=d_t[:], in0=d_t[:], scalar1=1.0,
                            scalar2=None, op0=ALU.is_ge)
    nc.vector.tensor_add(out=i0f_t[:], in0=i0f_t[:], in1=d_t[:])
    nc.vector.tensor_scalar(out=i0f_t[:], in0=i0f_t[:], scalar1=6.0,
                            scalar2=0.0, op0=ALU.min, op1=ALU.max)
    # interleaved (1-w, w) pairs: wall[..., 1] = w = (i_shift + 0.5) - i0,
    # wall[..., 0] = 1 - w
    wall_t = spool.tile([128, NG, 3, 2], F32, tag="wall")
    nc.vector.scalar_tensor_tensor(out=wall_t[:, :, :, 1], in0=ish_t[:],
                                   scalar=0.5, in1=i0f_t[:],
                                   op0=ALU.add, op1=ALU.subtract)
    nc.vector.tensor_scalar(out=wall_t[:, :, :, 0], in0=wall_t[:, :, :, 1],
                            scalar1=-1.0, scalar2=1.0,
                            op0=ALU.mult, op1=ALU.add)
    # base row of the point's cell, then all 8 corner rows in one add
    # (crb holds rowbase + 64*cz + 8*cy + xb per corner, built on host)
    pr_t = spool.tile([128, NG, 3], F32, tag="pr")
    nc.vector.tensor_mul(out=pr_t[:], in0=i0f_t[:],
                         in1=mul3_v.unsqueeze(1).to_broadcast([128, NG, 3]))
    ib_t = spool.tile([128, NG, 1], F32, tag="ib")
    nc.vector.reduce_sum(out=ib_t[:], in_=pr_t[:], axis=mybir.AxisListType.X)
    idx8f_t = spool.tile([128, NG, 8], F32, tag="idx8f")
    nc.vector.tensor_add(out=idx8f_t[:],
                         in0=ib_t[:].to_broadcast([128, NG, 8]), in1=crb_v)
    idx_t = spool.tile([128, NG, 8], I32, tag="idx")
    nc.vector.tensor_copy(out=idx_t[:], in_=idx8f_t[:])
    # corner weights wc[..., (cz, cy), xb] = zsel * ysel * xsel via two
    # outer products over the interleaved pairs
    yz_t = spool.tile([128, NG, 2, 2], F32, tag="yz")
    nc.vector.tensor_mul(
        out=yz_t[:],
        in0=wall_t[:, :, 2, :].unsqueeze(3).to_broadcast([128, NG, 2, 2]),
        in1=wall_t[:, :, 1, :].unsqueeze(2).to_broadcast([128, NG, 2, 2]))
    wc_t = spool.tile([128, NG, 4, 2], F32, tag="wc")
    nc.vector.tensor_mul(
        out=wc_t[:],
        in0=yz_t[:].rearrange("p g a b -> p g (a b)").unsqueeze(3)
            .to_broadcast([128, NG, 4, 2]),
        in1=wall_t[:, :, 0, :].unsqueeze(2).to_broadcast([128, NG, 4, 2]))
    wc_t = wc_t[:].rearrange("p g a b -> p g (a b)")

    # ---- DMA ordering plan (everything contends for the same DMA
    # engines, so the issue order is sequenced with explicit deps):
    #   consts -> wkt+wqt -> group-0 gathers -> wvt+wot -> group-1
    #   gathers -> output writes.
    # wkt/wqt (needed first: K pass, scores) load during the dead time
    # before the gathers' indices are computed; wvt/wot wait until the
    # group-0 gathers are through; group 1's gathers yield to wvt/wot. ----
    w_tiles = {}
    w_dmas = {}
    bioT8 = None
    for name, ap, dt_ in (("wqt", wqt, FP8), ("wkt", wkt, BF16),
                          ("wvt", wvt, BF16), ("wot", wot, BF16)):
        t = wpool.tile([128, CH, E], dt_, tag=name)
        eng = nc.scalar if name in ("wkt", "wqt") else nc.sync
        d0 = eng.dma_start(out=t[:, 0:3, :], in_=ap[:, 0:3, :])
        d1 = eng.dma_start(out=t[:, 3:6, :], in_=ap[:, 3:6, :])
        w_tiles[name] = t
        w_dmas[name] = (d0, d1)
        if name == "wqt":
            # q's other operand, right behind wqt on the ACT ring
            bioT8 = cpool.tile([128, CH * BPC], FP8, tag="cb8")
            nc.scalar.dma_start(out=bioT8[:], in_=cb8[:])
    bioT8_v = bioT8[:].rearrange("p (c b) -> p c b", b=BPC)

    # ---- all 16 single-row gathers issued up front (qPoolDynamic) ----
    corner_tiles = {}
    gather_insts = {}
    for g in range(NG):
        for c8 in range(8):
            pt = gpool.tile([128, E], BF16, tag=f"corner{g}{c8}")
            gi = nc.gpsimd.indirect_dma_start(
                out=pt[:], out_offset=None, in_=x[:],
                in_offset=bass.IndirectOffsetOnAxis(
                    ap=idx_t[:, g, c8:c8 + 1], axis=0),
            )
            corner_tiles[(g, c8)] = pt
            gather_insts[(g, c8)] = gi

    # wvt/wot yield to group 0's gathers (the SP HWDGE ring is otherwise
    # free then, and the gathers are the front critical path). Group 1's
    # gathers flow right behind group 0's on the SWDGE rings — with x in
    # bf16 there is enough HBM headroom to overlap them with wvt/wot.
    for name in ("wvt", "wot"):
        for d in w_dmas[name]:
            add_dep_helper(d.ins, gather_insts[(0, 5)].ins,
                           reason="late weights wait for group-0 gathers")

    # ---- trilinear corner accumulate: per-corner in-place scales + DVE
    # add-tree. Group 0's scales run on ACT (its window is before the
    # drain/exp traffic; activation takes a per-partition AP scale);
    # group 1's run on DVE in 2x mode (443ns tensor_scalar) because by
    # then ACT is congested with drains. bf16 throughout (~1e-3 extra
    # rel err from 4 roundings).
    def mac_chain(acc, g, scale_eng):
        cs = []
        for c8 in range(8):
            ct = corner_tiles[(g, c8)]
            if scale_eng == "act":
                nc.scalar.activation(out=ct[:], in_=ct[:],
                                     func=ACTF.Identity, bias=0.0,
                                     scale=wc_t[:, g, c8:c8 + 1])
            else:
                nc.vector.tensor_scalar(out=ct[:], in0=ct[:],
                                        scalar1=wc_t[:, g, c8:c8 + 1],
                                        scalar2=None, op0=ALU.mult)
            cs.append(ct)
        for a, b in ((0, 1), (2, 3), (4, 5), (6, 7), (0, 2), (4, 6)):
            nc.vector.tensor_add(out=cs[a][:], in0=cs[a][:], in1=cs[b][:])
        nc.vector.tensor_add(out=acc[:], in0=cs[0][:], in1=cs[4][:])

    acc0 = spool.tile([128, E], BF16, tag="acc0", name="acc0")
    mac_chain(acc0, 0, "act")
    acc1 = spool.tile([128, E], BF16, tag="acc1", name="acc1")
    acc_g = [acc0, acc1]

    # ---- q projection (all 8 batches): qT[co] = (Wq @ bio^T + bq) / 8.
    # Emitted first in the PE stream: PE is idle until the transposes are
    # ready, and q only depends on wqt + the bioT const. ----
    qT = []
    for co in range(CH):
        ps = psum([128, BPC])
        for t8 in range(CH // 2):
            nc.tensor.matmul(
                out=ps[:],
                lhsT=w_tiles["wqt"][:, 2 * t8:2 * t8 + 2,
                                    128 * co:128 * (co + 1)],
                rhs=bioT8_v[:, 2 * t8:2 * t8 + 2, :],
                start=(t8 == 0), stop=(t8 == CH // 2 - 1), perf_mode=DR)
        qt = cpool.tile([128, BPC], BF16, tag=f"qT{co}", name=f"qT{co}")
        nc.scalar.activation(out=qt[:], in_=ps[:], func=ACTF.Identity,
                             bias=bq_v[:, co:co + 1], scale=0.125 / WSCALE)
        qT.append(qt)

    # qexp after group 0's MAC in the DVE stream (scores need it later);
    # head columns padded to 32 so the scores psum blocks land 32-aligned
    qexp = []
    for ci in range(CH):
        qe = cpool.tile([128, BPC, NHP], BF16, tag=f"qexp{ci}",
                        name=f"qexp{ci}")
        nc.vector.tensor_mul(
            out=qe[:],
            in0=qT[ci][:].unsqueeze(2).to_broadcast([128, BPC, NHP]),
            in1=hsel_v[:, ci, :].unsqueeze(1).to_broadcast([128, BPC, NHP]))
        qexp.append(qe)

    boc_g = []
    for gg in range(NG):
        bc_ = spool.tile([GB, E], F32, tag=f"boc{gg}", name=f"boc{gg}")
        nc.vector.tensor_scalar(out=bc_[:], in0=bo_v,
                                scalar1=conf_v[:, gg:gg + 1],
                                scalar2=None, op0=ALU.mult)
        boc_g.append(bc_)

    # ---- per-group pipeline ----
    for g in range(NG):
        acc = acc_g[g]

        # transpose to channel-major bf16 (PSUM drain copies split between
        # ACT and DVE so neither serializes the chain)
        sampT = spool.tile([128, CH, 128], BF16, tag=f"sampT{g}",
                           name=f"sampT{g}")
        for ci in range(CH):
            ps = psum([128, 128], dt=BF16)
            nc.tensor.transpose(
                out=ps[:], in_=acc[:, 128 * ci:128 * (ci + 1)],
                identity=iden_v)
            if ci % 2 == 0:
                nc.scalar.copy(out=sampT[:, ci, :], in_=ps[:])
            else:
                nc.vector.tensor_copy(out=sampT[:, ci, :], in_=ps[:])

        # K / V projections (weights pre-folded with sample_proj)
        def proj_pass(wname, bias_v, out_tag):
            outs = []
            for co in range(CH):
                ps = psum([128, 128])
                for ci in range(CH):
                    nc.tensor.matmul(
                        out=ps[:],
                        lhsT=w_tiles[wname][:, ci, 128 * co:128 * (co + 1)],
                        rhs=sampT[:, ci, :],
                        start=(ci == 0), stop=(ci == CH - 1))
                o = spool.tile([128, 128], BF16, tag=f"{out_tag}{g}{co}",
                               name=f"{out_tag}{g}{co}")
                if co % 2 == 0:
                    nc.scalar.activation(out=o[:], in_=ps[:],
                                         func=ACTF.Identity,
                                         bias=bias_v[:, co:co + 1], scale=1.0)
                else:
                    nc.vector.tensor_scalar(out=o[:], in0=ps[:],
                                            scalar1=bias_v[:, co:co + 1],
                                            scalar2=None, op0=ALU.add)
                outs.append(o)
            return outs

        kT = proj_pass("wkt", bk_v, "kT")

        # scores, all 4 batches in one matmul chain: lhsT packs the group's
        # (batch, head) q columns -> psum [48, 128]; only the block-diagonal
        # (b == b') strips are real, the off-diagonal 3/4 is discarded. 6
        # matmuls replace 24 tiny ones (PE instruction overhead dominated).
        sc_ps = psum([GB * NHP, GB * NB])
        qsl = slice(GB * g, GB * (g + 1))
        for ci in range(CH):
            nc.tensor.matmul(
                out=sc_ps[:],
                lhsT=qexp[ci][:, qsl, :].rearrange("p b h -> p (b h)"),
                rhs=kT[ci][:], start=(ci == 0), stop=(ci == CH - 1))

        # softmax over points. Scores here are bounded (|s| < ~1: q is
        # pre-scaled by 1/8 and both operands are O(0.3)-scale random
        # projections), so exp() is computed without the max-subtraction.
        ex_t = spool.tile([NH, GB, NB], F32, tag=f"ex{g}", name=f"ex{g}")
        for b in range(GB):
            nc.scalar.activation(out=ex_t[:, b, :],
                                 in_=sc_ps[NHP * b:NHP * b + NH,
                                           NB * b:NB * (b + 1)],
                                 func=ACTF.Exp)
        s_t = spool.tile([NH, GB, 1], F32, tag=f"sm{g}", name=f"sm{g}")
        nc.vector.reduce_sum(out=s_t[:], in_=ex_t[:],
                             axis=mybir.AxisListType.X)
        r_t = spool.tile([NH, GB], F32, tag=f"rc{g}", name=f"rc{g}")
        nc.vector.reciprocal(out=r_t[:], in_=s_t[:, :, 0])
        at_t = spool.tile([NH, GB, NB], BF16, tag=f"attn{g}", name=f"attn{g}")
        nc.vector.tensor_mul(out=at_t[:], in0=ex_t[:],
                             in1=r_t[:].unsqueeze(2).to_broadcast(
                                 [NH, GB, NB]))

        # V projection after the scores/softmax (its weights arrive later)
        vT = proj_pass("wvt", bv_v, "vT")

        if g == 0:
            # group 1's corner accumulation, emitted after group 0's V
            # drains: its gathers are done by then, and emitting later
            # strands acc1 behind group 0's full ACT drain traffic
            mac_chain(acc1, 1, "act")

        # broadcast attn rows to channel layout; ctx reduction (DVE reads
        # the PSUM product input directly; no staging copy)
        ctxF = spool.tile([128, CH, GB], F32, tag=f"ctxF{g}", name=f"ctxF{g}")
        for ci in range(CH):
            ps = psum([128, GB * NB])
            nc.tensor.matmul(
                out=ps[:], lhsT=bsel_t[:, 128 * ci:128 * (ci + 1)],
                rhs=at_t[:], start=True, stop=True)
            prod = spool.tile([128, GB, NB], F32, tag=f"prod{g}{ci}",
                              name=f"prod{g}{ci}")
            nc.vector.tensor_mul(
                out=prod[:],
                in0=vT[ci][:].rearrange("p (b n) -> p b n", n=NB),
                in1=ps[:].rearrange("p (b n) -> p b n", n=NB))
            nc.vector.reduce_sum(out=ctxF[:, ci, :].unsqueeze(2),
                                 in_=prod[:], axis=mybir.AxisListType.X)
        ctxT = spool.tile([128, CH, GB], BF16, tag=f"ctxT{g}", name=f"ctxT{g}")
        nc.vector.tensor_copy(out=ctxT[:], in_=ctxF[:])

        # out projection + bias + confidence: outfin = ps*conf + bo*conf
        outfin = spool.tile([GB, E], BF16, tag=f"outfin{g}", name=f"outfin{g}")
        for half in range(2):
            sl = slice(384 * half, 384 * (half + 1))
            ps = psum([GB, 384], opp)
            for ci in range(CH):
                nc.tensor.matmul(
                    out=ps[:], lhsT=ctxT[:, ci, :],
                    rhs=w_tiles["wot"][:, ci, sl],
                    start=(ci == 0), stop=(ci == CH - 1))
            nc.vector.scalar_tensor_tensor(
                out=outfin[:, sl], in0=ps[:],
                scalar=conf_v[:, g:g + 1],
                in1=boc_g[g][:][:, sl],
                op0=ALU.mult, op1=ALU.add)

        # broadcast each batch row to 128 partitions; write 512 rows per
        # batch, alternating between the SP and ACT HWDGE rings so the four
        # transfers pipeline two-wide. The 513th rows of the whole group go
        # out as ONE [4, E] strided DMA straight from outfin (saves four
        # fixed-cost-dominated single-row DMAs).
        for b in range(GB):
            bb = GB * g + b
            bt = bcpool.tile([128, E], BF16, tag="bt", name=f"bt{bb}")
            for half in range(2):
                sl = slice(384 * half, 384 * (half + 1))
                ps = psum([128, 384], opp)
                nc.tensor.matmul(
                    out=ps[:], lhsT=oneh_t[:, 128 * b:128 * (b + 1)],
                    rhs=outfin[:, sl], start=True, stop=True)
                if (b + half) % 2 == 0:
                    nc.scalar.copy(out=bt[:, sl], in_=ps[:])
                else:
                    nc.vector.tensor_copy(out=bt[:, sl], in_=ps[:])
            r0 = FULLN * bb
            dst = out[r0:r0 + 512, :].rearrange("(p f) e -> p f e", f=4)
            src = bt[:].unsqueeze(1).to_broadcast([128, 4, E])
            eng = nc.sync if b % 2 == 0 else nc.scalar
            if b == 0:
                # the group's first write: two half-width DMAs, each gated
                # only on its own staging copy, so the first transfer's
                # HWDGE issue overlaps the second half's copy
                for half in range(2):
                    sl = slice(384 * half, 384 * (half + 1))
                    eng.dma_start(out=dst[:, :, sl], in_=src[:, :, sl])
            else:
                eng.dma_start(out=dst, in_=src)
        tail_dst = out.rearrange("(b r) e -> b r e", r=FULLN)[
            GB * g:GB * (g + 1), FULLN - 1, :]
        nc.sync.dma_start(out=tail_dst, in_=outfin[:])


_NO_SPLIT_TYPES = {"InstUnconditionalBranch", "InstConditionalBranch"}


def _split_waits(nc, max_waits=1):
    # walrus (CoreV3) accepts only one sync-wait command per compute
    # instruction; move extra waits onto injected same-engine NoOps placed
    # immediately before the instruction (semantics unchanged).
    import bass_rust
    k = 0
    for fn in nc.m.functions:
        for bb in fn.blocks:
            insts = bb.instructions
            i = 0
            while i < len(insts):
                inst = insts[i]
                si = inst.sync_info
                if (type(inst).__name__ not in _NO_SPLIT_TYPES
                        and si is not None
                        and si.on_wait and len(si.on_wait) > max_waits):
                    waits = list(si.on_wait)
                    extra, keep = waits[:-max_waits], waits[-max_waits:]
                    for w in extra:
                        k += 1
                        nop = bass_rust.InstNoOp(name=f"I-wsplit-{k}",
                                                 engine=inst.engine,
                                                 ins=[], outs=[])
                        nop.sync_info = bass_rust.SyncInfo(on_wait=[w],
                                                           on_update=[])
                        insts.insert(i, nop)
                        i += 1
                    inst.sync_info = bass_rust.SyncInfo(
                        on_wait=keep, on_update=list(si.on_update or []))
                i += 1
    return k


def build(split=True):
    from contextlib import ExitStack

    # 48KB SWDGE descriptor carveout: all 16 indirect gathers (129 descs
    # each) fit in flight at once; the default 16KB ring caps ~8 and
    # trickles the second group's gathers behind the first's completions.
    nc = bass.Bass("TRN2", debug=False, num_devices=NCORES,
                   dynamic_dma_scratch_size=16384)
    with tile.TileContext(nc) as tc, ExitStack() as es:
        _body(es, tc)
    if split:
        # needed for the walrus compile; CoreSim can't replay injected nops
        _split_waits(nc)
    return nc


def host_prep(inputs):
    """Build per-core in_maps from full inputs (layout marshalling + weight
    folding/casting only)."""
    x = np.ascontiguousarray(inputs["x"], dtype=np.float32)
    bio = np.ascontiguousarray(inputs["bio_embed"], dtype=np.float32)
    base = np.ascontiguousarray(inputs["base_coords"], dtype=np.float32)
    offsets = np.ascontiguousarray(inputs["offsets"], dtype=np.float32)
    confidence = np.ascontiguousarray(inputs["confidence"], dtype=np.float32)
    wsp = np.asarray(inputs["sample_proj_w"], dtype=np.float32)
    bsp = np.asarray(inputs["sample_proj_b"], dtype=np.float32)
    win = np.asarray(inputs["in_proj_w"], dtype=np.float32)
    bin_ = np.asarray(inputs["in_proj_b"], dtype=np.float32)
    wout = np.asarray(inputs["out_proj_w"], dtype=np.float32)
    bout = np.asarray(inputs["out_proj_b"], dtype=np.float32)

    # fold sample_proj into Wk / Wv (exact algebra, done in f64 on host)
    wk, wv = win[E:2 * E], win[2 * E:]
    bkf = wk @ bsp + bin_[E:2 * E]
    bvf = wv @ bsp + bin_[2 * E:]
    wks = (wk.astype(np.float64) @ wsp.astype(np.float64)).astype(np.float32)
    wvs = (wv.astype(np.float64) @ wsp.astype(np.float64)).astype(np.float32)

    def chunkT(w, dt=ml_dtypes.bfloat16):  # [E, E] -> [128, CH, E] of w^T
        return np.ascontiguousarray(
            w.T.reshape(CH, 128, E).transpose(1, 0, 2)).astype(dt)

    _FP8 = ml_dtypes.float8_e4m3

    def chunkT8(w):  # fp8, host-scaled by WSCALE to dodge e4m3 subnormals
        return chunkT(w * WSCALE, _FP8)

    def chunkb(v):  # [E] -> [128, CH]
        return np.ascontiguousarray(v.reshape(CH, 128).T)

    # f32 [128, *] const blob
    cfb = np.zeros((128, _CF_COLS), np.float32)
    cfb[:, _C_BASE:_C_OFFS] = np.tile(base, (BPC, 1)).reshape(
        NG, 128, 3).transpose(1, 0, 2).reshape(128, NG * 3)
    rowb = ((np.arange(ROWS) // NB) * FULLN + 1.0).astype(
        np.float32).reshape(NG, 128).T  # [128, NG]
    coff = np.array([64 * cz + 8 * cy + xb
                     for (cz, cy) in ((0, 0), (0, 1), (1, 0), (1, 1))
                     for xb in (0, 1)], np.float32)  # [8]
    cfb[:, _C_CRB:_C_MUL3] = (rowb[:, :, None] + coff[None, None, :]).reshape(
        128, NG * 8)
    cfb[:, _C_MUL3:_C_HEAD] = np.tile(
        np.array([1.0, 8.0, 64.0], np.float32), (128, 1))
    cfb[:, _C_BQ:_C_BK] = chunkb(bin_[:E] * 0.125)
    cfb[:, _C_BK:_C_BV] = chunkb(bkf)
    cfb[:, _C_BV:_CF_COLS] = chunkb(bvf)

    # bf16 [128, *] const blob (bioT filled per core below)
    cbb = np.zeros((128, _BF_COLS), np.float32)
    hsel = np.zeros((128, CH, NHP), np.float32)
    for ch in range(CH):
        for p in range(128):
            hsel[p, ch, (ch * 128 + p) // HD] = 1.0
    cbb[:, _B_HSEL:_B_IDEN] = hsel.reshape(128, CH * NHP)
    cbb[:, _B_IDEN:_BF_COLS] = np.eye(128, dtype=np.float32)

    bsel = np.zeros((NH, CH * 128), np.float32)
    for ch in range(CH):
        for j in range(128):
            bsel[(ch * 128 + j) // HD, ch * 128 + j] = 1.0
    oneh = np.zeros((GB, GB * 128), np.float32)
    for b in range(GB):
        oneh[b, 128 * b:128 * (b + 1)] = 1.0

    consts = {
        "wqt": chunkT8(win[:E]),
        "wkt": chunkT(wks),
        "wvt": chunkT(wvs),
        "wot": chunkT(wout),
        "cf": cfb,
        "bsel": bsel.astype(ml_dtypes.bfloat16),
        "onehg": oneh.astype(ml_dtypes.bfloat16),
    }

    in_maps = []
    for c in range(NCORES):
        bsl = slice(BPC * c, BPC * (c + 1))
        bio_c = bio[bsl]  # [8, 768]
        m = dict(consts)
        m["x"] = x[bsl].reshape(BPC * FULLN, E).astype(ml_dtypes.bfloat16)
        cfc = cfb.copy()
        cfc[:, _C_OFFS:_C_CRB] = offsets[bsl].reshape(
            NG, 128, 3).transpose(1, 0, 2).reshape(128, NG * 3)
        m["cf"] = cfc
        cbc = cbb.copy()
        bioT = bio_c.T.reshape(CH, 128, BPC).transpose(
            1, 0, 2).reshape(128, CH * BPC)
        cbc[:, _B_BIOT:_B_HSEL] = bioT
        m["cb"] = cbc.astype(ml_dtypes.bfloat16)
        m["cb8"] = bioT.astype(_FP8)
        pfb = np.zeros((GB, _PF_COLS), np.float32)
        pfb[:, _P_CONF:_P_BO] = confidence[bsl].reshape(NG, GB).T
        pfb[:, _P_BO:_PF_COLS] = bout[None, :]
        m["pf"] = pfb
        in_maps.append(m)
    return in_maps


_NC = None


def kernel(**inputs):
    global _NC
    if _NC is None:
        _NC = build()
    in_maps = host_prep(inputs)
    res = bass_utils.run_bass_kernel_spmd(_NC, in_maps,
                                          core_ids=list(range(NCORES)))
    outs = [res.results[c]["out"].reshape(BPC, FULLN, E).astype(np.float32)
            for c in range(NCORES)]
    return np.concatenate(outs, axis=0)

